# revision 4
# baseline (speedup 1.0000x reference)
"""RBF kernel matrix on 8 Trainium2 NeuronCores.

out[i, j] = exp(-||x_i - y_j||^2), x: [8192, 64], y: [8192, 64], f32.

Strategy (pure data-parallel over rows of x, per the sharding hint):
each core c gets x rows [c*1024, (c+1)*1024) plus a full replica of y and
produces the [1024, 8192] output slab.  No collectives.

Per-core math:  s = 2*x.y - ||x||^2 - ||y||^2  (= -squared distance), then
out = exp(s).  The GEMM computes  s + x2  directly via one augmented fp32
matmul per tile:
    lhsT_aug [65, 128] = [2*x^T ; ones]
    rhs_aug  [65, 512] = [y^T   ; -y2 ]
so psum = 2*x.y - y2.  The remaining  -x2  rides the free per-partition bias
of the ScalarE activation:  out = Exp(psum + (-x2)).
"""

import numpy as np

_N = 8192
_M = 8192
_D = 64
_NC = 8
_ROWS = _N // _NC  # 1024 rows of x per core
_P = 128

_compiled = None


def _build_nc():
    from contextlib import ExitStack

    import concourse.bacc as bacc
    import concourse.tile as tile
    from concourse import masks, mybir

    f32 = mybir.dt.float32
    Exp = mybir.ActivationFunctionType.Exp
    AX = mybir.AxisListType.X
    ADD = mybir.AluOpType.add

    NT = _ROWS // _P  # 8 m-tiles of 128 rows
    NCH = _M // 512  # 16 column chunks of 512

    nc = bacc.Bacc("TRN2", target_bir_lowering=False, debug=False, num_devices=_NC)
    x_in = nc.declare_dram_parameter("x_slab", [_ROWS, _D], f32, isOutput=False)
    y_in = nc.declare_dram_parameter("y", [_M, _D], f32, isOutput=False)
    out = nc.declare_dram_parameter("out", [_ROWS, _M], f32, isOutput=True)

    with tile.TileContext(nc) as tc, ExitStack() as ctx:
        singles = ctx.enter_context(tc.tile_pool(name="singles", bufs=1))
        ident = singles.tile([_P, _P], f32)
        masks.make_identity(nc, ident[:])
        ones_col = singles.tile([_D, 1], f32)
        nc.vector.memset(ones_col[:], 1.0)

        # Persistent operands for the whole kernel
        xT = singles.tile([_D + 1, _ROWS], f32)  # rows 0-63: 2*x^T, row 64: 1
        rhs = singles.tile([_D + 1, _M], f32)  # rows 0-63: y^T, row 64: -y2
        negx2 = singles.tile([_P, NT], f32)
        xnat = singles.tile([_P, NT, _D], f32)
        xsq = singles.tile([_P, NT, _D], f32)
        x2 = singles.tile([_P, NT], f32)

        with ExitStack() as pro:
            ptrx = pro.enter_context(tc.tile_pool(name="ptrx", bufs=2, space="PSUM"))
            ptry = pro.enter_context(tc.tile_pool(name="ptry", bufs=2, space="PSUM"))
            py2p = pro.enter_context(tc.tile_pool(name="py2", bufs=2, space="PSUM"))
            ynat_pool = pro.enter_context(tc.tile_pool(name="ynat", bufs=3))
            ysq_pool = pro.enter_context(tc.tile_pool(name="ysq", bufs=3))

            # ---- x side: load, x2, transpose ----
            nc.sync.dma_start(xnat[:], x_in[:, :].rearrange("(t p) d -> p t d", p=_P))
            nc.vector.tensor_mul(xsq[:], xnat[:], xnat[:])
            for t in range(NT):
                nc.vector.tensor_reduce(
                    out=x2[:, t : t + 1], in_=xsq[:, t, :], axis=AX, op=ADD
                )
            nc.vector.tensor_scalar_mul(negx2[:], x2[:], -1.0)
            for t in range(NT):
                pt = ptrx.tile([_D, _P], f32, tag="ptrx")
                nc.tensor.transpose(pt[:], xnat[:, t, :], ident[:])
                nc.scalar.mul(xT[0:_D, t * _P : (t + 1) * _P], pt[:], 2.0)
            nc.vector.memset(xT[_D : _D + 1, :], 1.0)

            # ---- y side, per 512-row chunk: load, transpose, y2 ----
            for ch in range(NCH):
                sl = slice(ch * 512, (ch + 1) * 512)
                ynat = ynat_pool.tile([_P, 4, _D], f32)
                nc.sync.dma_start(
                    ynat[:], y_in[sl, :].rearrange("(t p) d -> p t d", p=_P)
                )
                pt = ptry.tile([_D, 512], f32, tag="ptry")
                for u in range(4):
                    nc.tensor.transpose(
                        pt[:, u * _P : (u + 1) * _P], ynat[:, u, :], ident[:]
                    )
                nc.vector.tensor_copy(rhs[0:_D, sl], pt[:])
                ysq = ysq_pool.tile([_D, 512], f32)
                nc.vector.tensor_mul(ysq[:], rhs[0:_D, sl], rhs[0:_D, sl])
                py2 = py2p.tile([1, 512], f32)
                nc.tensor.matmul(py2[:], ones_col[:], ysq[:], start=True, stop=True)
                nc.scalar.mul(rhs[_D : _D + 1, sl], py2[:], -1.0)

        # ---- main loop: matmul -> exp -> store ----
        pmm = ctx.enter_context(tc.tile_pool(name="pmm", bufs=2, space="PSUM"))
        outp = ctx.enter_context(tc.tile_pool(name="outp", bufs=3))
        for t in range(NT):
            lhs = xT[:, t * _P : (t + 1) * _P]
            for g in range(4):
                pg = pmm.tile([_P, 2048], f32)
                for u in range(4):
                    csl = slice((g * 4 + u) * 512, (g * 4 + u + 1) * 512)
                    nc.tensor.matmul(
                        pg[:, u * 512 : (u + 1) * 512],
                        lhs,
                        rhs[:, csl],
                        start=True,
                        stop=True,
                    )
                ot = outp.tile([_P, 2048], f32)
                nc.scalar.activation(
                    ot[:], pg[:], Exp, bias=negx2[:, t : t + 1], scale=1.0
                )
                nc.sync.dma_start(
                    out[t * _P : (t + 1) * _P, g * 2048 : (g + 1) * 2048], ot[:]
                )

    nc.compile()
    return nc


def _get_compiled():
    global _compiled
    if _compiled is None:
        _compiled = _build_nc()
    return _compiled


def kernel(x: np.ndarray, y: np.ndarray, _trace: bool = False):
    from concourse.bass_utils import run_bass_kernel_spmd

    x = np.ascontiguousarray(np.asarray(x, dtype=np.float32))
    y = np.ascontiguousarray(np.asarray(y, dtype=np.float32))
    assert x.shape == (_N, _D) and y.shape == (_M, _D)

    nc = _get_compiled()
    in_maps = [
        {"x_slab": x[c * _ROWS : (c + 1) * _ROWS], "y": y} for c in range(_NC)
    ]
    res = run_bass_kernel_spmd(nc, in_maps, list(range(_NC)), trace=_trace)
    out = np.concatenate([r["out"] for r in res.results], axis=0)
    if _trace:
        kernel.last_results = res
    return out


# revision 6
# speedup vs baseline: 1.4615x; 1.4615x over previous
"""RBF kernel matrix on 8 Trainium2 NeuronCores.

out[i, j] = exp(-||x_i - y_j||^2), x: [8192, 64], y: [8192, 64], f32.

Sharding (per the hint): core c gets x rows [c*1024, (c+1)*1024) plus a full
replica of y and produces its [1024, 8192] output slab. No communication.

Per-core math:  s = a.y - x2 - y2  with  a = 2x,  then out = exp(s).
The GEMM runs as two fp16 hi/lo passes accumulating in fp32 PSUM (fp32
matmuls run at 1/4 rate AND never un-throttle the PE clock):
  pass1 [66,128]x[66,512]:   [a_hi; 1; 1] . [y_hi; -y2_hi; -y2_lo]
  pass2 [128,128]x[128,512]: [a_lo; a_hi] . [y_hi; y_lo]
so psum = a.y - y2 to ~1e-7.  The remaining -x2 rides the free per-partition
bias of the ScalarE Exp activation.
"""

import numpy as np

_N = 8192
_M = 8192
_D = 64
_NC = 8
_ROWS = _N // _NC  # 1024 rows of x per core
_P = 128

_compiled = None


def _build_nc():
    from contextlib import ExitStack

    import concourse.bacc as bacc
    import concourse.tile as tile
    from concourse import masks, mybir

    f32 = mybir.dt.float32
    f16 = mybir.dt.float16
    Exp = mybir.ActivationFunctionType.Exp
    AX = mybir.AxisListType.X
    ADD = mybir.AluOpType.add

    NT = _ROWS // _P  # 8 m-tiles of 128 rows
    NCH = _M // 512  # 16 column chunks of 512

    nc = bacc.Bacc("TRN2", target_bir_lowering=False, debug=False, num_devices=_NC)
    x_in = nc.declare_dram_parameter("x_slab", [_ROWS, _D], f32, isOutput=False)
    y_in = nc.declare_dram_parameter("y", [_M, _D], f32, isOutput=False)
    out = nc.declare_dram_parameter("out", [_ROWS, _M], f32, isOutput=True)

    with tile.TileContext(nc) as tc, ExitStack() as ctx:
        singles = ctx.enter_context(tc.tile_pool(name="singles", bufs=1))
        ident = singles.tile([_P, _P], f32)
        masks.make_identity(nc, ident[:])
        negones = singles.tile([_D, 1], f32)
        nc.vector.memset(negones[:], -1.0)

        # Persistent operands.  Aug rows live at partitions 64 and 96 (engine
        # APs must start at a multiple of 32); rows 65-95 are zero on both
        # sides so they contribute nothing.
        xa_hi = singles.tile([97, _ROWS], f16)  # [a_hi ; 1 ; 0.. ; 1]
        xa_cat = singles.tile([_P, _ROWS], f16)  # [a_lo ; a_hi]
        y_aug = singles.tile([97, _M], f16)  # [y_hi ; -y2_hi ; 0.. ; -y2_lo]
        y_cat = singles.tile([_P, _M], f16)  # [y_hi ; y_lo]
        negx2 = singles.tile([_P, NT], f32)
        xnat = singles.tile([_P, NT, _D], f32)
        xsq = singles.tile([_P, NT, _D], f32)
        x2 = singles.tile([_P, NT], f32)
        xTf = singles.tile([_D, _ROWS], f32)  # a = 2*x^T, fp32

        with ExitStack() as pro:
            ptrx = pro.enter_context(tc.tile_pool(name="ptrx", bufs=2, space="PSUM"))
            ptry = pro.enter_context(tc.tile_pool(name="ptry", bufs=2, space="PSUM"))
            py2p = pro.enter_context(tc.tile_pool(name="py2", bufs=2, space="PSUM"))
            ynat_pool = pro.enter_context(tc.tile_pool(name="ynat", bufs=3))
            ytf_pool = pro.enter_context(tc.tile_pool(name="ytf", bufs=3))
            ysq_pool = pro.enter_context(tc.tile_pool(name="ysq", bufs=3))

            # ---- x side: load, x2, transpose, hi/lo split ----
            nc.sync.dma_start(xnat[:], x_in[:, :].rearrange("(t p) d -> p t d", p=_P))
            nc.vector.tensor_mul(xsq[:], xnat[:], xnat[:])
            for t in range(NT):
                nc.vector.tensor_reduce(
                    out=x2[:, t : t + 1], in_=xsq[:, t, :], axis=AX, op=ADD
                )
            nc.vector.tensor_scalar_mul(negx2[:], x2[:], -1.0)
            for t in range(NT):
                pt = ptrx.tile([_D, _P], f32, tag="ptrx")
                nc.tensor.transpose(pt[:], xnat[:, t, :], ident[:])
                nc.scalar.mul(xTf[:, t * _P : (t + 1) * _P], pt[:], 2.0)
            nc.vector.memset(xa_hi[:, :], 0.0)
            nc.vector.memset(y_aug[:, :], 0.0)
            nc.vector.tensor_copy(xa_hi[0:_D, :], xTf[:])
            nc.vector.memset(xa_hi[_D : _D + 1, :], 1.0)
            nc.vector.memset(xa_hi[96:97, :], 1.0)
            nc.vector.tensor_copy(xa_cat[_D : 2 * _D, :], xa_hi[0:_D, :])
            nc.vector.tensor_sub(xa_cat[0:_D, :], xTf[:], xa_hi[0:_D, :])

            # ---- y side, per 512-row chunk ----
            for ch in range(NCH):
                sl = slice(ch * 512, (ch + 1) * 512)
                ynat = ynat_pool.tile([_P, 4, _D], f32)
                nc.sync.dma_start(
                    ynat[:], y_in[sl, :].rearrange("(t p) d -> p t d", p=_P)
                )
                pt = ptry.tile([_D, 512], f32, tag="ptry")
                for u in range(4):
                    nc.tensor.transpose(
                        pt[:, u * _P : (u + 1) * _P], ynat[:, u, :], ident[:]
                    )
                ytf = ytf_pool.tile([_D, 512], f32)
                nc.scalar.copy(ytf[:], pt[:])
                # hi/lo split (y_hi exact in fp16+residual)
                nc.vector.tensor_copy(y_cat[0:_D, sl], ytf[:])
                nc.vector.tensor_copy(y_aug[0:_D, sl], y_cat[0:_D, sl])
                nc.vector.tensor_sub(y_cat[_D : 2 * _D, sl], ytf[:], y_cat[0:_D, sl])
                # y2 = sum_k yT^2 : fp32 colsum via PE; psum gets -y2
                ysq = ysq_pool.tile([_D, 512], f32)
                nc.vector.tensor_mul(ysq[:], ytf[:], ytf[:])
                py2 = py2p.tile([1, 512], f32)
                nc.tensor.matmul(py2[:], negones[:], ysq[:], start=True, stop=True)
                nc.vector.tensor_copy(y_aug[_D : _D + 1, sl], py2[:])
                nc.vector.tensor_sub(
                    y_aug[96:97, sl], py2[:], y_aug[_D : _D + 1, sl]
                )

        # ---- main loop: 2-pass matmul -> exp -> store ----
        pmm = ctx.enter_context(tc.tile_pool(name="pmm", bufs=2, space="PSUM"))
        outp = ctx.enter_context(tc.tile_pool(name="outp", bufs=3))
        for t in range(NT):
            tsl = slice(t * _P, (t + 1) * _P)
            for g in range(4):
                pg = pmm.tile([_P, 2048], f32)
                for u in range(4):
                    csl = slice((g * 4 + u) * 512, (g * 4 + u + 1) * 512)
                    nc.tensor.matmul(
                        pg[:, u * 512 : (u + 1) * 512],
                        xa_hi[:, tsl],
                        y_aug[:, csl],
                        start=True,
                        stop=False,
                    )
                for u in range(4):
                    csl = slice((g * 4 + u) * 512, (g * 4 + u + 1) * 512)
                    nc.tensor.matmul(
                        pg[:, u * 512 : (u + 1) * 512],
                        xa_cat[:, tsl],
                        y_cat[:, csl],
                        start=False,
                        stop=True,
                    )
                if g % 2 == 0:
                    ot = outp.tile([_P, 4096], f32)
                nc.scalar.activation(
                    ot[:, (g % 2) * 2048 : (g % 2 + 1) * 2048],
                    pg[:],
                    Exp,
                    bias=negx2[:, t : t + 1],
                    scale=1.0,
                )
                if g % 2 == 1:
                    nc.sync.dma_start(
                        out[tsl, (g - 1) * 2048 : (g + 1) * 2048], ot[:]
                    )

    nc.compile()
    return nc


def _get_compiled():
    global _compiled
    if _compiled is None:
        _compiled = _build_nc()
    return _compiled


def kernel(x: np.ndarray, y: np.ndarray, _trace: bool = False):
    from concourse.bass_utils import run_bass_kernel_spmd

    x = np.ascontiguousarray(np.asarray(x, dtype=np.float32))
    y = np.ascontiguousarray(np.asarray(y, dtype=np.float32))
    assert x.shape == (_N, _D) and y.shape == (_M, _D)

    nc = _get_compiled()
    in_maps = [
        {"x_slab": x[c * _ROWS : (c + 1) * _ROWS], "y": y} for c in range(_NC)
    ]
    res = run_bass_kernel_spmd(nc, in_maps, list(range(_NC)), trace=_trace)
    out = np.concatenate([r["out"] for r in res.results], axis=0)
    if _trace:
        kernel.last_results = res
    return out


# revision 7
# speedup vs baseline: 2.0565x; 1.4071x over previous
"""RBF kernel matrix on 8 Trainium2 NeuronCores.

out[i, j] = exp(-||x_i - y_j||^2), x: [8192, 64], y: [8192, 64], f32.

Sharding (per the hint): core c gets x rows [c*1024, (c+1)*1024) plus a full
replica of y and produces its [1024, 8192] output slab. No communication.

Per-core math:  s = a.y - x2 - y2  with  a = 2x,  then out = exp(s).
The GEMM runs as two fp16 hi/lo passes accumulating in fp32 PSUM (fp32
matmuls run at 1/4 rate AND never un-throttle the PE clock):
  pass1 [97,128]x[97,512]:   [a_hi; 1; 0..; 1] . [y_hi; -y2_hi; 0..; -y2_lo]
  pass2 [128,128]x[128,512]: [a_lo; a_hi] . [y_hi; y_lo]
so psum = a.y - y2 to ~1e-7.  The remaining -x2 rides the free per-partition
bias of the ScalarE Exp activation.  (Aug rows sit at partitions 64 and 96
because engine APs must start at a multiple of 32; the rows between are zero
on both sides.)

Loop order is column-group-major (g outer, m-tile inner) so the first output
stores issue as soon as the first 4 y-chunks are prepped; the 32 MiB store
stream is the roofline and runs continuously from ~15us.
"""

import numpy as np

_N = 8192
_M = 8192
_D = 64
_NC = 8
_ROWS = _N // _NC  # 1024 rows of x per core
_P = 128

_compiled = None


def _build_nc():
    from contextlib import ExitStack

    import concourse.bacc as bacc
    import concourse.tile as tile
    from concourse import masks, mybir

    f32 = mybir.dt.float32
    f16 = mybir.dt.float16
    Exp = mybir.ActivationFunctionType.Exp
    AX = mybir.AxisListType.X
    ADD = mybir.AluOpType.add

    NT = _ROWS // _P  # 8 m-tiles of 128 rows
    NCH = _M // 512  # 16 column chunks of 512
    NG = 4  # column groups of 2048

    nc = bacc.Bacc("TRN2", target_bir_lowering=False, debug=False, num_devices=_NC)
    x_in = nc.declare_dram_parameter("x_slab", [_ROWS, _D], f32, isOutput=False)
    y_in = nc.declare_dram_parameter("y", [_M, _D], f32, isOutput=False)
    out = nc.declare_dram_parameter("out", [_ROWS, _M], f32, isOutput=True)

    with tile.TileContext(nc) as tc, ExitStack() as ctx:
        singles = ctx.enter_context(tc.tile_pool(name="singles", bufs=1))
        ident = singles.tile([_P, _P], f32)
        masks.make_identity(nc, ident[:])
        negones = singles.tile([_D, 1], f32)
        nc.vector.memset(negones[:], -1.0)

        # Persistent operands
        xa_hi = singles.tile([97, _ROWS], f16)  # [a_hi ; 1 ; 0.. ; 1]
        xa_cat = singles.tile([_P, _ROWS], f16)  # [a_lo ; a_hi]
        y_aug = singles.tile([97, _M], f16)  # [y_hi ; -y2_hi ; 0.. ; -y2_lo]
        y_cat = singles.tile([_P, _M], f16)  # [y_hi ; y_lo]
        negx2 = singles.tile([_P, NT], f32)
        xnat = singles.tile([_P, NT, _D], f32)
        xsq = singles.tile([_P, NT, _D], f32)
        x2 = singles.tile([_P, NT], f32)
        xTf = singles.tile([_D, _ROWS], f32)  # a = 2*x^T, fp32

        # PSUM: prologue pools and the main pool use disjoint banks so main
        # matmuls never WAR-wait on prologue psum reuse.
        ptr = ctx.enter_context(tc.tile_pool(name="ptr", bufs=2, space="PSUM"))
        py2p = ctx.enter_context(tc.tile_pool(name="py2", bufs=2, space="PSUM"))
        pmm = ctx.enter_context(tc.tile_pool(name="pmm", bufs=2, space="PSUM"))

        ynat_pool = ctx.enter_context(tc.tile_pool(name="ynat", bufs=4))
        ytf_pool = ctx.enter_context(tc.tile_pool(name="ytf", bufs=4))
        ysq_pool = ctx.enter_context(tc.tile_pool(name="ysq", bufs=4))
        outp = ctx.enter_context(tc.tile_pool(name="outp", bufs=4))

        # ---- x side: load, x2, transpose, hi/lo split ----
        nc.sync.dma_start(xnat[:], x_in[:, :].rearrange("(t p) d -> p t d", p=_P))
        nc.vector.tensor_mul(xsq[:], xnat[:], xnat[:])
        for t in range(NT):
            nc.vector.tensor_reduce(
                out=x2[:, t : t + 1], in_=xsq[:, t, :], axis=AX, op=ADD
            )
        nc.vector.tensor_scalar_mul(negx2[:], x2[:], -1.0)
        for t in range(NT):
            pt = ptr.tile([_D, 512], f32, tag="ptr")
            nc.tensor.transpose(pt[:, 0:_P], xnat[:, t, :], ident[:])
            nc.scalar.mul(xTf[:, t * _P : (t + 1) * _P], pt[:, 0:_P], 2.0)
        nc.gpsimd.memset(xa_hi[:, :], 0.0)
        nc.gpsimd.memset(y_aug[:, :], 0.0)
        nc.vector.tensor_copy(xa_hi[0:_D, :], xTf[:])
        nc.vector.memset(xa_hi[_D : _D + 1, :], 1.0)
        nc.vector.memset(xa_hi[96:97, :], 1.0)
        nc.vector.tensor_copy(xa_cat[_D : 2 * _D, :], xa_hi[0:_D, :])
        nc.vector.tensor_sub(xa_cat[0:_D, :], xTf[:], xa_hi[0:_D, :])

        # ---- y chunk prep (emitted first; scheduler interleaves with main) ----
        def prep_chunk(ch):
            sl = slice(ch * 512, (ch + 1) * 512)
            ynat = ynat_pool.tile([_P, 4, _D], f32)
            nc.sync.dma_start(
                ynat[:], y_in[sl, :].rearrange("(t p) d -> p t d", p=_P)
            )
            pt = ptr.tile([_D, 512], f32, tag="ptr")
            for u in range(4):
                nc.tensor.transpose(
                    pt[:, u * _P : (u + 1) * _P], ynat[:, u, :], ident[:]
                )
            ytf = ytf_pool.tile([_D, 512], f32)
            nc.vector.tensor_copy(ytf[:], pt[:])
            # hi/lo split
            nc.vector.tensor_copy(y_cat[0:_D, sl], ytf[:])
            nc.vector.tensor_copy(y_aug[0:_D, sl], y_cat[0:_D, sl])
            nc.vector.tensor_sub(y_cat[_D : 2 * _D, sl], ytf[:], y_cat[0:_D, sl])
            # -y2 = -sum_k yT^2 via PE colsum with -1 weights
            ysq = ysq_pool.tile([_D, 512], f32)
            nc.vector.tensor_mul(ysq[:], ytf[:], ytf[:])
            py2 = py2p.tile([1, 512], f32)
            nc.tensor.matmul(py2[:], negones[:], ysq[:], start=True, stop=True)
            nc.vector.tensor_copy(y_aug[_D : _D + 1, sl], py2[:])
            nc.vector.tensor_sub(y_aug[96:97, sl], py2[:], y_aug[_D : _D + 1, sl])

        for ch in range(NCH):
            prep_chunk(ch)

        # ---- main: column-group outer, m-tile inner ----
        for g in range(NG):
            for t in range(NT):
                tsl = slice(t * _P, (t + 1) * _P)
                ot = outp.tile([_P, 2048], f32)
                for h in range(2):  # two psum halves of 1024
                    pg = pmm.tile([_P, 1024], f32)
                    for u in range(2):
                        chn = g * 4 + h * 2 + u
                        csl = slice(chn * 512, (chn + 1) * 512)
                        nc.tensor.matmul(
                            pg[:, u * 512 : (u + 1) * 512],
                            xa_hi[:, tsl],
                            y_aug[:, csl],
                            start=True,
                            stop=False,
                        )
                    for u in range(2):
                        chn = g * 4 + h * 2 + u
                        csl = slice(chn * 512, (chn + 1) * 512)
                        nc.tensor.matmul(
                            pg[:, u * 512 : (u + 1) * 512],
                            xa_cat[:, tsl],
                            y_cat[:, csl],
                            start=False,
                            stop=True,
                        )
                    nc.scalar.activation(
                        ot[:, h * 1024 : (h + 1) * 1024],
                        pg[:],
                        Exp,
                        bias=negx2[:, t : t + 1],
                        scale=1.0,
                    )
                nc.sync.dma_start(
                    out[tsl, g * 2048 : (g + 1) * 2048], ot[:]
                )

    nc.compile()
    return nc


def _get_compiled():
    global _compiled
    if _compiled is None:
        _compiled = _build_nc()
    return _compiled


def kernel(x: np.ndarray, y: np.ndarray, _trace: bool = False):
    from concourse.bass_utils import run_bass_kernel_spmd

    x = np.ascontiguousarray(np.asarray(x, dtype=np.float32))
    y = np.ascontiguousarray(np.asarray(y, dtype=np.float32))
    assert x.shape == (_N, _D) and y.shape == (_M, _D)

    nc = _get_compiled()
    in_maps = [
        {"x_slab": x[c * _ROWS : (c + 1) * _ROWS], "y": y} for c in range(_NC)
    ]
    res = run_bass_kernel_spmd(nc, in_maps, list(range(_NC)), trace=_trace)
    out = np.concatenate([r["out"] for r in res.results], axis=0)
    if _trace:
        kernel.last_results = res
    return out


# revision 11
# speedup vs baseline: 2.0900x; 1.0163x over previous
"""RBF kernel matrix on 8 Trainium2 NeuronCores.

out[i, j] = exp(-||x_i - y_j||^2), x: [8192, 64], y: [8192, 64], f32.

Sharding (per the hint): core c gets x rows [c*1024, (c+1)*1024) plus a full
replica of y and produces its [1024, 8192] output slab. No communication.

Per-core math:  s = a.y - x2 - y2  with  a = 2x,  then out = exp(s).
The GEMM runs as two fp16 hi/lo passes accumulating in fp32 PSUM (fp32
matmuls run at 1/4 rate AND never un-throttle the PE clock):
  pass1 [97,128]x[97,512]:   [a_hi; 1; 0..; 1] . [y_hi; -y2_hi; 0..; -y2_lo]
  pass2 [128,128]x[128,512]: [a_lo; a_hi] . [y_hi; y_lo]
so psum = a.y - y2 to ~1e-7.  The remaining -x2 rides the free per-partition
bias of the ScalarE Exp activation.  (Aug rows sit at partitions 64 and 96
because engine APs must start at a multiple of 32; the rows between are zero
on both sides.)

Loop order is column-group-major (g outer, m-tile inner) so the first output
stores issue as soon as the first 4 y-chunks are prepped; the 32 MiB store
stream is the roofline and runs continuously from ~15us.
"""

import numpy as np

_N = 8192
_M = 8192
_D = 64
_NC = 8
_ROWS = _N // _NC  # 1024 rows of x per core
_P = 128

_compiled = None


def _build_nc():
    from contextlib import ExitStack

    import concourse.bacc as bacc
    import concourse.tile as tile
    from concourse import masks, mybir

    f32 = mybir.dt.float32
    f16 = mybir.dt.float16
    Exp = mybir.ActivationFunctionType.Exp
    AX = mybir.AxisListType.X
    ADD = mybir.AluOpType.add

    NT = _ROWS // _P  # 8 m-tiles of 128 rows
    NCH = _M // 512  # 16 column chunks of 512
    NG = 4  # column groups of 2048

    nc = bacc.Bacc("TRN2", target_bir_lowering=False, debug=False, num_devices=_NC)
    x_in = nc.declare_dram_parameter("x_slab", [_ROWS, _D], f32, isOutput=False)
    y_in = nc.declare_dram_parameter("y", [_M, _D], f32, isOutput=False)
    out = nc.declare_dram_parameter("out", [_ROWS, _M], f32, isOutput=True)

    with tile.TileContext(nc) as tc, ExitStack() as ctx:
        singles = ctx.enter_context(tc.tile_pool(name="singles", bufs=1))
        ident = singles.tile([_P, _P], f32)
        masks.make_identity(nc, ident[:])
        negones = singles.tile([_D, 1], f32)
        nc.vector.memset(negones[:], -1.0)

        # PE warmup: HAM only un-throttles (1.2->2.4 GHz) after ~3.4us of
        # sustained matmul activity, and neither transposes nor fp32 matmuls
        # count as busy.  Burn ~4us of junk fp16 matmuls first so the whole
        # prologue runs warm.
        warm_in = singles.tile([_P, 512], f16)
        nc.gpsimd.memset(warm_in[:, :], 0.0)

        # Persistent operands
        xa_hi = singles.tile([97, _ROWS], f16)  # [a_hi ; 1 ; 0.. ; 1]
        xa_cat = singles.tile([_P, _ROWS], f16)  # [a_lo ; a_hi]
        y_aug = singles.tile([97, _M], f16)  # [y_hi ; -y2_hi ; 0.. ; -y2_lo]
        y_cat = singles.tile([_P, _M], f16)  # [y_hi ; y_lo]
        negx2 = singles.tile([_P, NT], f32)
        xnat = singles.tile([_P, NT, _D], f32)
        xsq = singles.tile([_P, NT, _D], f32)
        x2 = singles.tile([_P, NT], f32)
        xTf = singles.tile([_D, _ROWS], f32)  # a = 2*x^T, fp32

        # PSUM: prologue pools and the main pool use disjoint banks so main
        # matmuls never WAR-wait on prologue psum reuse.
        ptr = ctx.enter_context(tc.tile_pool(name="ptr", bufs=2, space="PSUM"))
        py2p = ctx.enter_context(tc.tile_pool(name="py2", bufs=2, space="PSUM"))
        pmm = ctx.enter_context(tc.tile_pool(name="pmm", bufs=2, space="PSUM"))

        ynat_pool = ctx.enter_context(tc.tile_pool(name="ynat", bufs=4))
        ytf_pool = ctx.enter_context(tc.tile_pool(name="ytf", bufs=4))
        ysq_pool = ctx.enter_context(tc.tile_pool(name="ysq", bufs=4))
        outp = ctx.enter_context(tc.tile_pool(name="outp", bufs=6))

        wp = ptr.tile([_P, 512], f32, tag="ptr")
        for _ in range(12):
            nc.tensor.matmul(
                wp[:], warm_in[:, 0:_P], warm_in[:], start=True, stop=True
            )

        # ---- x side: load, x2, transpose, hi/lo split ----
        nc.sync.dma_start(xnat[:], x_in[:, :].rearrange("(t p) d -> p t d", p=_P))
        nc.vector.tensor_mul(xsq[:], xnat[:], xnat[:])
        for t in range(NT):
            nc.vector.tensor_reduce(
                out=x2[:, t : t + 1], in_=xsq[:, t, :], axis=AX, op=ADD
            )
        nc.vector.tensor_scalar_mul(negx2[:], x2[:], -1.0)
        for t in range(NT):
            pt = ptr.tile([_D, 512], f32, tag="ptr")
            nc.tensor.transpose(pt[:, 0:_P], xnat[:, t, :], ident[:])
            nc.scalar.mul(xTf[:, t * _P : (t + 1) * _P], pt[:, 0:_P], 2.0)
        nc.gpsimd.memset(xa_hi[:, :], 0.0)
        nc.gpsimd.memset(y_aug[:, :], 0.0)
        nc.vector.tensor_copy(xa_hi[0:_D, :], xTf[:])
        nc.vector.memset(xa_hi[_D : _D + 1, :], 1.0)
        nc.vector.memset(xa_hi[96:97, :], 1.0)
        nc.vector.tensor_copy(xa_cat[_D : 2 * _D, :], xa_hi[0:_D, :])
        nc.vector.tensor_sub(xa_cat[0:_D, :], xTf[:], xa_hi[0:_D, :])

        # ---- y chunk prep (emitted first; scheduler interleaves with main) ----
        def prep_chunk(ch):
            sl = slice(ch * 512, (ch + 1) * 512)
            ynat = ynat_pool.tile([_P, 4, _D], f32)
            nc.sync.dma_start(
                ynat[:], y_in[sl, :].rearrange("(t p) d -> p t d", p=_P)
            )
            pt = ptr.tile([_D, 512], f32, tag="ptr")
            for u in range(4):
                nc.tensor.transpose(
                    pt[:, u * _P : (u + 1) * _P], ynat[:, u, :], ident[:]
                )
            ytf = ytf_pool.tile([_D, 512], f32)
            nc.vector.tensor_copy(ytf[:], pt[:])
            # hi/lo split
            nc.vector.tensor_copy(y_cat[0:_D, sl], ytf[:])
            nc.vector.tensor_copy(y_aug[0:_D, sl], y_cat[0:_D, sl])
            nc.vector.tensor_sub(y_cat[_D : 2 * _D, sl], ytf[:], y_cat[0:_D, sl])
            # -y2 = -sum_k yT^2 via PE colsum with -1 weights
            ysq = ysq_pool.tile([_D, 512], f32)
            nc.vector.tensor_mul(ysq[:], ytf[:], ytf[:])
            py2 = py2p.tile([1, 512], f32)
            nc.tensor.matmul(py2[:], negones[:], ysq[:], start=True, stop=True)
            nc.vector.tensor_copy(y_aug[_D : _D + 1, sl], py2[:])
            nc.vector.tensor_sub(y_aug[96:97, sl], py2[:], y_aug[_D : _D + 1, sl])

        for ch in range(NCH):
            prep_chunk(ch)

        # ---- main: column-group outer, m-tile inner ----
        for g in range(NG):
            for t in range(NT):
                tsl = slice(t * _P, (t + 1) * _P)
                ot = outp.tile([_P, 2048], f32)
                for h in range(2):  # two psum halves of 1024
                    pg = pmm.tile([_P, 1024], f32)
                    for u in range(2):
                        chn = g * 4 + h * 2 + u
                        csl = slice(chn * 512, (chn + 1) * 512)
                        nc.tensor.matmul(
                            pg[:, u * 512 : (u + 1) * 512],
                            xa_hi[:, tsl],
                            y_aug[:, csl],
                            start=True,
                            stop=False,
                        )
                    for u in range(2):
                        chn = g * 4 + h * 2 + u
                        csl = slice(chn * 512, (chn + 1) * 512)
                        nc.tensor.matmul(
                            pg[:, u * 512 : (u + 1) * 512],
                            xa_cat[:, tsl],
                            y_cat[:, csl],
                            start=False,
                            stop=True,
                        )
                    nc.scalar.activation(
                        ot[:, h * 1024 : (h + 1) * 1024],
                        pg[:],
                        Exp,
                        bias=negx2[:, t : t + 1],
                        scale=1.0,
                    )
                nc.sync.dma_start(
                    out[tsl, g * 2048 : (g + 1) * 2048], ot[:]
                )

    nc.compile()
    return nc


def _get_compiled():
    global _compiled
    if _compiled is None:
        _compiled = _build_nc()
    return _compiled


def kernel(x: np.ndarray, y: np.ndarray, _trace: bool = False):
    from concourse.bass_utils import run_bass_kernel_spmd

    x = np.ascontiguousarray(np.asarray(x, dtype=np.float32))
    y = np.ascontiguousarray(np.asarray(y, dtype=np.float32))
    assert x.shape == (_N, _D) and y.shape == (_M, _D)

    nc = _get_compiled()
    in_maps = [
        {"x_slab": x[c * _ROWS : (c + 1) * _ROWS], "y": y} for c in range(_NC)
    ]
    res = run_bass_kernel_spmd(nc, in_maps, list(range(_NC)), trace=_trace)
    out = np.concatenate([r["out"] for r in res.results], axis=0)
    if _trace:
        kernel.last_results = res
    return out
